# revision 2
# baseline (speedup 1.0000x reference)
"""Trainium2 Bass kernel for nn_ConstrainedEnhancementModel (v2).

Model: x(512,256,32) -> flatten(512,8192) -> MLP encoder/decoder
(8192->1024->512->256->512->1024->131072) -> constraint blend with
linearly-interpolated low-res data.

Host-side fold (as v1): the post-matmul constraint
   out = where(knot, x_knot, where(in_seg, 0.8*interp + 0.2*decoded, decoded))
is linear in x, so it folds into the final projection: scale w6/b6
columns by 0 (knot), 0.2 (in-segment) or 1.0 (tail); per 512-column
segment a K=65 interp matmul block (x_start rows, x_end rows, ones row
carrying b6_eff) adds the interpolation + bias term.

v2 changes vs v1 (396us):
 - Transposed output orientation: psum tiles are [cols, batch]; the
   moving operand of the big projection is d2 in its natural
   [feature, batch] layout and the interp bias rides in the stationary
   coefficient block. Output DRAM is [COLS, B] per core, bf16
   (host transposes back); halves the dominant output write traffic.
 - All encoder weights/activations bf16 (same PE rate, half the DMA).
 - The 1024x16384-per-core projection runs fp8e4 with
   perf_mode=DoubleRow (virtual K=256 per MM -> 4 MMs instead of 8 per
   psum tile, ~1.4x tensor-engine throughput; w6 prescaled by SW=512 to
   clear the fp8 subnormal floor, undone in the psum->sbuf copy).
   Measured end-to-end rel err of this scheme (numpy emulation): 3.8e-3.
 - AllReduce payload bf16 (half the collective traffic).

Sharding unchanged: encoder K-sharded L1 + AllReduce, replicated L2-L5,
big projection column-sharded (core c owns output timesteps
[c*512, (c+1)*512)).
"""

from contextlib import ExitStack

import numpy as np
import ml_dtypes

import concourse.bacc as bacc
import concourse.mybir as mybir
import concourse.tile as tile
from concourse.bass import ds, ts
from concourse.bass_utils import run_bass_kernel_spmd

DT = mybir.dt

B, L, F, H, HID = 512, 256, 32, 4096, 512
UP = H // L          # 16 timesteps per low-res segment
LF = L * F           # 8192
HF = H * F           # 131072
NCORES = 8
COLS = HF // NCORES  # 16384 output rows per core (outT orientation)
SEGC = UP * F        # 512 output rows per segment
NSEG = COLS // SEGC  # 32 segments per core
KI = 2 * F + 1       # 65: interp-block contraction size
NBIAS = 8 + 4 + 2 + 4 + 8  # packed bias columns
SW = 512.0           # w6/interp prescale (fp8 subnormal floor), undone on copy-out

_CACHE: dict = {}


def _build_program(reps=1, phase="all"):
    """One SPMD program; per-core data differences live in the inputs.

    reps>1 repeats the whole body back-to-back inside one NEFF (timing).
    phase: "all" | "enc" (encoder only) | "big" (projection only, dummy d2).
    """
    bf16, f32, f8 = DT.bfloat16, DT.float32, DT.float8e4
    nc = bacc.Bacc("TRN2", target_bir_lowering=False, debug=False, num_devices=NCORES)

    KSH = LF // NCORES  # 1024 contraction rows of layer 1 per core
    xTs = nc.dram_tensor("xTs", [KSH, B], bf16, kind="ExternalInput")
    w1s = nc.dram_tensor("w1s", [KSH, 2 * HID], bf16, kind="ExternalInput")
    arin = nc.dram_tensor("arin", [2 * HID, B], bf16)
    arout = nc.dram_tensor("arout", [2 * HID, B], bf16, addr_space="Shared")
    w2 = nc.dram_tensor("w2", [2 * HID, HID], bf16, kind="ExternalInput")
    w3 = nc.dram_tensor("w3", [HID, HID // 2], bf16, kind="ExternalInput")
    w4 = nc.dram_tensor("w4", [HID // 2, HID], bf16, kind="ExternalInput")
    w5 = nc.dram_tensor("w5", [HID, 2 * HID], bf16, kind="ExternalInput")
    bpk = nc.dram_tensor("bpk", [128, NBIAS], f32, kind="ExternalInput")
    w6dr = nc.dram_tensor("w6dr", [NSEG, 128, 4, 4, 2, 128], f8, kind="ExternalInput")
    cxk = nc.dram_tensor("cxk", [NSEG, KI, 2, SEGC], bf16, kind="ExternalInput")
    out = nc.dram_tensor("out", [COLS, B], bf16, kind="ExternalOutput")

    RELU = mybir.ActivationFunctionType.Relu
    IDENT = mybir.ActivationFunctionType.Identity
    DR = mybir.MatmulPerfMode.DoubleRow

    with tile.TileContext(nc) as tc:

        def _one_rep(rep, ctx):
            psum = ctx.enter_context(
                tc.tile_pool(name=f"psum{rep}", bufs=8, space="PSUM")
            )
            xpool = ctx.enter_context(tc.tile_pool(name=f"xpool{rep}", bufs=4))
            wpool = ctx.enter_context(tc.tile_pool(name=f"wpool{rep}", bufs=5))
            scratch = ctx.enter_context(tc.tile_pool(name=f"scratch{rep}", bufs=2))
            acts = ctx.enter_context(tc.tile_pool(name=f"acts{rep}", bufs=1))
            bpool = ctx.enter_context(tc.tile_pool(name=f"bpool{rep}", bufs=1))
            w6pool = ctx.enter_context(tc.tile_pool(name=f"w6pool{rep}", bufs=3))
            cxpool = ctx.enter_context(tc.tile_pool(name=f"cxpool{rep}", bufs=3))
            opool = ctx.enter_context(tc.tile_pool(name=f"opool{rep}", bufs=3))

            btile = bpool.tile([128, NBIAS], f32, name="btile")
            nc.scalar.dma_start(btile[:], bpk[:])
            boff = {1: 0, 2: 8, 3: 12, 4: 14, 5: 18}

            def _enc():
                # ---- L1: K-sharded partial matmul + AllReduce over 8 cores ----
                ps1 = [
                    psum.tile([128, B], f32, tag="psum", name=f"ps1_{m}")
                    for m in range(8)
                ]
                for kc in range(4):
                    e1 = nc.sync if kc % 2 == 0 else nc.scalar
                    e2 = nc.scalar if kc % 2 == 0 else nc.sync
                    xt = xpool.tile([128, 2, B], bf16, name=f"xt{kc}", tag="xt")
                    e2.dma_start(
                        xt[:],
                        xTs[ds(kc * 256, 256), :].rearrange("(k p) d -> p k d", p=128),
                    )
                    w1t = wpool.tile([128, 2, 2 * HID], bf16, name=f"w1t{kc}", tag="w")
                    e1.dma_start(
                        w1t[:],
                        w1s[ds(kc * 256, 256), :].rearrange("(k p) d -> p k d", p=128),
                    )
                    for k4 in range(2):
                        for m in range(8):
                            nc.tensor.matmul(
                                ps1[m][:],
                                w1t[:, k4, ts(m, 128)],
                                xt[:, k4, :],
                                start=(kc == 0 and k4 == 0),
                                stop=(kc == 3 and k4 == 1),
                            )
                hp = scratch.tile([128, 8, B], bf16, tag="s", name="hp")
                for m in range(8):
                    nc.vector.tensor_copy(hp[:, m, :], ps1[m][:])
                nc.sync.dma_start(
                    arin[ds(0, 512), :].rearrange("(m p) d -> p m d", p=128),
                    hp[:, 0:4, :],
                )
                nc.scalar.dma_start(
                    arin[ds(512, 512), :].rearrange("(m p) d -> p m d", p=128),
                    hp[:, 4:8, :],
                )
                nc.gpsimd.collective_compute(
                    "AllReduce",
                    mybir.AluOpType.add,
                    replica_groups=[list(range(NCORES))],
                    ins=[arin[:]],
                    outs=[arout[:]],
                )
                htmp = scratch.tile([128, 8, B], bf16, tag="s", name="htmp")
                nc.sync.dma_start(
                    htmp[:, 0:4, :],
                    arout[ds(0, 512), :].rearrange("(m p) d -> p m d", p=128),
                )
                nc.scalar.dma_start(
                    htmp[:, 4:8, :],
                    arout[ds(512, 512), :].rearrange("(m p) d -> p m d", p=128),
                )
                h1 = scratch.tile([128, 8, B], bf16, tag="s", name="h1")
                for m in range(8):
                    nc.scalar.activation(
                        h1[:, m, :], htmp[:, m, :], RELU, bias=btile[:, m : m + 1]
                    )

                # ---- L2..L5 (one DMA per layer, weights via shared pool) ----
                def mlp_layer(w_dram, k_tiles, m_tiles, rhs, b_idx, func, name, pool,
                              out_dtype=bf16):
                    o = pool.tile(
                        [128, m_tiles, B], out_dtype,
                        tag="s" if pool is scratch else name, name=name,
                    )
                    ps = [
                        psum.tile([128, B], f32, tag="psum", name=f"ps_{name}_{m}")
                        for m in range(m_tiles)
                    ]
                    for kc in range(0, k_tiles, 2):
                        kw = min(2, k_tiles - kc)
                        wt = wpool.tile(
                            [128, kw, m_tiles * 128], bf16, tag="w",
                            name=f"w_{name}_{kc}",
                        )
                        eng = nc.sync if (kc // 2) % 2 == 0 else nc.scalar
                        eng.dma_start(
                            wt[:],
                            w_dram[ds(kc * 128, kw * 128), :].rearrange(
                                "(k p) d -> p k d", p=128
                            ),
                        )
                        for ki in range(kw):
                            for m in range(m_tiles):
                                nc.tensor.matmul(
                                    ps[m][:],
                                    wt[:, ki, ts(m, 128)],
                                    rhs[:, kc + ki, :],
                                    start=(kc + ki == 0),
                                    stop=(kc + ki == k_tiles - 1),
                                )
                    ob = boff[b_idx]
                    for m in range(m_tiles):
                        nc.scalar.activation(
                            o[:, m, :], ps[m][:], func,
                            bias=btile[:, ob + m : ob + m + 1],
                        )
                    return o

                h2 = mlp_layer(w2, 8, 4, h1, 2, RELU, "h2", scratch)
                ft = mlp_layer(w3, 4, 2, h2, 3, IDENT, "ft", scratch)
                d1 = mlp_layer(w4, 2, 4, ft, 4, RELU, "d1", scratch)
                return mlp_layer(w5, 4, 8, d1, 5, RELU, "d2", acts, out_dtype=f8)

            def _big(d2):
                # ---- big projection (fp8 DoubleRow) + folded interp, 32 segs ----
                inv = 1.0 / SW
                for s in range(NSEG):
                    cx = cxpool.tile([KI, 2, SEGC], bf16, name=f"cx{s}", tag="cx")
                    nc.sync.dma_start(cx[:], cxk[s])
                    w6t = w6pool.tile([128, 4, 4, 2, 128], f8, name=f"w6t{s}", tag="w6")
                    nc.sync.dma_start(w6t[:], w6dr[s])
                    obuf = opool.tile([128, 4, SEGC], bf16, tag="ot", name=f"ot{s}")
                    for jt in range(4):
                        ps = psum.tile([128, SEGC], f32, tag="psum", name=f"pso_{s}_{jt}")
                        nc.tensor.matmul(
                            ps[:],
                            cx[:, 0, ts(jt, 128)],
                            cx[:, 1, :],
                            start=True,
                            stop=False,
                        )
                        for kt in range(4):
                            nc.tensor.matmul(
                                ps[:],
                                w6t[:, jt, kt, :, :],
                                d2[:, ds(2 * kt, 2), :],
                                perf_mode=DR,
                                start=False,
                                stop=(kt == 3),
                            )
                        if jt % 2 == 0:
                            nc.scalar.activation(
                                obuf[:, jt, :], ps[:], IDENT, scale=inv
                            )
                        else:
                            nc.vector.tensor_scalar_mul(obuf[:, jt, :], ps[:], inv)
                    nc.scalar.dma_start(
                        out[ds(s * SEGC, SEGC), :].rearrange("(a p) d -> p a d", p=128),
                        obuf[:],
                    )

            if phase == "enc":
                d2 = _enc()
                otx = opool.tile([128, B], bf16, name="otx", tag="ot")
                nc.vector.tensor_copy(otx[:], d2[:, 0, :])
                nc.sync.dma_start(out[ts(0, 128), :], otx[:])
            elif phase == "big":
                d2 = acts.tile([128, 8, B], f8, tag="d2", name="d2")
                nc.vector.memset(d2[:], 0.5)
                _big(d2)
            else:
                _big(_enc())

        for _rep in range(reps):
            with ExitStack() as _ctx:
                _one_rep(_rep, _ctx)

    nc.compile()
    return nc


def _host_prep(inputs):
    """Shard + fold + quantize. Returns per-core input maps."""
    BF = ml_dtypes.bfloat16
    F8 = ml_dtypes.float8_e4m3

    x = np.ascontiguousarray(inputs["low_res_data"], dtype=np.float32)
    x2d = x.reshape(B, LF)
    xTa = np.ascontiguousarray(x2d.T)  # (8192, 512)
    w6 = np.asarray(inputs["w6"], dtype=np.float32)
    b6 = np.asarray(inputs["b6"], dtype=np.float32)

    # per-output-column scale: 0 on knots, 0.2 in-segment, 1.0 in the tail
    h = np.arange(H)
    colscale = np.where(h % UP == 0, 0.0, np.where(h < (L - 1) * UP, 0.2, 1.0))
    colscale = np.repeat(colscale, F).astype(np.float32)  # (HF,)
    b6_eff = b6 * colscale

    # interp coefficient blocks (shared by all segments except the last)
    fidx = np.arange(F)
    std = np.zeros((KI, SEGC), np.float32)
    last = np.zeros((KI, SEGC), np.float32)
    for h_off in range(UP):
        a = h_off / UP
        cs = 1.0 if h_off == 0 else 0.8 * (1.0 - a)
        ce = 0.0 if h_off == 0 else 0.8 * a
        std[fidx, h_off * F + fidx] = cs
        std[F + fidx, h_off * F + fidx] = ce
        last[fidx, h_off * F + fidx] = 1.0 if h_off == 0 else 0.0

    bpk = np.zeros((128, NBIAS), np.float32)
    off = 0
    for i in (1, 2, 3, 4, 5):
        bv = np.asarray(inputs[f"b{i}"], np.float32)
        m = bv.shape[0] // 128
        bpk[:, off : off + m] = bv.reshape(m, 128).T
        off += m

    w1f = np.asarray(inputs["w1"], np.float32)
    shared = {
        "w2": np.asarray(inputs["w2"], np.float32).astype(BF),
        "w3": np.asarray(inputs["w3"], np.float32).astype(BF),
        "w4": np.asarray(inputs["w4"], np.float32).astype(BF),
        "w5": np.asarray(inputs["w5"], np.float32).astype(BF),
        "bpk": bpk,
    }

    in_maps = []
    for c in range(NCORES):
        j0 = c * COLS
        # DoubleRow pack: w6dr[s, p, jt, kt, i, j] = w6_eff[256kt+128i+p, 512s+128jt+j]*SW
        w6c = np.ascontiguousarray(w6[:, j0 : j0 + COLS]) * (
            colscale[j0 : j0 + COLS] * SW
        )
        arr = w6c.reshape(4, 2, 128, NSEG, 4, 128)  # [kt, i, p, s, jt, j]
        w6drc = np.ascontiguousarray(arr.transpose(3, 2, 4, 0, 1, 5)).astype(F8)

        xTsl = np.ascontiguousarray(
            xTa[c * (LF // NCORES) : (c + 1) * (LF // NCORES)]
        ).astype(BF)
        w1sl = np.ascontiguousarray(
            w1f[c * (LF // NCORES) : (c + 1) * (LF // NCORES)]
        ).astype(BF)

        cxk = np.zeros((NSEG, KI, 2, SEGC), np.float32)
        for sl in range(NSEG):
            s = c * NSEG + sl
            cxk[sl, :, 0, :] = (std if s < L - 1 else last) * SW
            cxk[sl, 2 * F, 0, :] = b6_eff[s * SEGC : (s + 1) * SEGC] * SW
            cxk[sl, 0:F, 1, :] = xTa[s * F : (s + 1) * F]
            if s + 1 < L:
                cxk[sl, F : 2 * F, 1, :] = xTa[(s + 1) * F : (s + 2) * F]
            cxk[sl, 2 * F, 1, :] = 1.0
        in_maps.append(
            {
                **shared,
                "w6dr": w6drc,
                "cxk": cxk.astype(BF),
                "xTs": xTsl,
                "w1s": w1sl,
            }
        )
    return in_maps


def kernel(**inputs):
    if "nc" not in _CACHE:
        _CACHE["nc"] = _build_program()
    nc = _CACHE["nc"]
    in_maps = _host_prep(inputs)
    res = run_bass_kernel_spmd(nc, in_maps, list(range(NCORES)))
    out = np.empty((B, H, F), np.float32)
    for c in range(NCORES):
        oc = np.asarray(res.results[c]["out"]).astype(np.float32)  # (COLS, B)
        out[:, c * (H // NCORES) : (c + 1) * (H // NCORES), :] = oc.T.reshape(
            B, H // NCORES, F
        )
    return out
